# revision 46
# baseline (speedup 1.0000x reference)
"""Trainium2 Bass kernel for nn_GRUCell_21612275433682.

Math (from the reference):
  - h0 = 0, so the W_hh matmul is dead: only b_hh enters the gates.
  - y = x @ W_ih.T            (the single big GEMM, [B*T, I] @ [I, 3H])
  - r = (y_r + b_ih_r + b_hh_r > 0)
  - z = (y_z + b_ih_z + b_hh_z > 0)
  - n = (y_n + b_ih_n + r*b_hh_n > 0)
  - cur = (1-z)*n   in {0,1}
  - LIF over T=4 steps:  mem' = 0.99*mem + cur_t - spk_{t-1};  spk_t = (mem' > 1)
    spk_0 is identically 0 (mem1 = cur0 <= 1).

Strategy: pure data parallel over 8 cores (B sharded 256/core). Per core one
[3H=6144, TB=1024] x [I=2048] GEMM with W stationary ([I,3H] tiles) and X
moving. Output layout [3h partitions, b-major (b*4+t) free] so biases are
per-partition scalars and the LIF is pure free-dim slicing.

GEMM precision schemes (SCHEME):
  - "f32r":    single fp32r ("replicated fp32") pass per gate.  Measured on
               HW: ~227ns per [128x128]x[128x512] tile (vs fp16's 216) with
               ~2x lower error than fp16 (y-rms 8.7e-5 vs 1.7e-4 at K=2048)
               and fp32 data straight from HBM (no host-side splitting).
               The PE quantizes each operand to ~11-12 mantissa bits
               internally (verified: 11-bit values pass through exactly;
               host pre-rounding cannot reduce the total error).
  - "bf16x3":  W,X split into bf16 hi/lo; 3 bf16 passes.
  - "f16f8":   fp16 hi pass + both cross terms packed into one fp8e4m3
               DoubleRow pass (measured: the DR pass costs a FULL fp16-pass
               equivalent, ~230ns, not the 0.5x the cost model claims).

The LIF scan collapses to booleans (cur in {0,1}, threshold 1, beta 0.99,
and one decay step never drops a positive residue below the spike gap):
  s1 = c0*c1, s2 = c2*[c0+c1>0], s3 = c3*[c0+c1+c2>0]; s0 identically 0.
This is exact vs the reference fp32 scan (the only boundary case, mem==1.0,
is exact in fp32 and the reference compares strictly).
"""

import numpy as np
import ml_dtypes

BF16 = ml_dtypes.bfloat16
FP8 = ml_dtypes.float8_e4m3

# Full problem sizes (hardcoded per contract)
B, I, H, T = 2048, 2048, 2048, 4
NCORES = 8
P = 128

SCHEME = "f32r"

# scheme f16f8 scale choices (powers of two; see product-scale table below)
#   main:  (wh * 2^8) @ (xh * 2^8)            -> y_main * 2^16
#   cross: fp8(wh*2^5) @ fp8(xl*2^11)         -> cross1 * 2^16
#          fp8(wl*2^16) @ fp8(xh)             -> cross2 * 2^16
SW_H, SX_H = 256.0, 256.0
SW8_H, SX8_L = 32.0, 2048.0
SW8_L, SX8_H = 65536.0, 1.0
SCALE = 65536.0

_CACHE = {}

# test-harness knobs (grading path leaves these alone)
TRACE = False
LAST_EXEC_NS = None
LAST_RESULTS = None


def _common_io(nc, mybir, KT, GJ, TB, scheme):
    f32 = mybir.dt.float32
    br_d = nc.dram_tensor("br", [P, GJ], f32, kind="ExternalInput")
    bz_d = nc.dram_tensor("bz", [P, GJ], f32, kind="ExternalInput")
    bin_d = nc.dram_tensor("bin", [P, GJ], f32, kind="ExternalInput")
    bhn_d = nc.dram_tensor("bhn", [P, GJ], f32, kind="ExternalInput")
    if scheme == "f32r":
        # b-major columns: out[j, n, p, t, bb]
        NT = TB // 512
        BN = TB // 4 // NT
        out_d = nc.dram_tensor("out", [GJ, NT, P, 3, BN], f32,
                               kind="ExternalOutput")
    else:
        out_d = nc.dram_tensor("out", [GJ, P, 3 * (TB // 4)], f32,
                               kind="ExternalOutput")
    return br_d, bz_d, bin_d, bhn_d, out_d


def build_nc(KT, GJ, BT, scheme=None):
    """Build the per-core Bass program.

    KT: number of 128-wide K tiles (I = 128*KT)
    GJ: number of 128-row h-tile groups per gate (H = 128*GJ)
    BT: batch rows per timestep per core (TB = 4*BT total moving columns)
    """
    import concourse.mybir as mybir
    import concourse.tile as tile
    from concourse import bacc

    scheme = scheme or SCHEME
    TB = 4 * BT
    NT = TB // 512
    assert NT * 512 == TB

    f32 = mybir.dt.float32
    bf16 = mybir.dt.bfloat16
    f16 = mybir.dt.float16
    f8 = mybir.dt.float8e4
    A = mybir.AluOpType
    DR = mybir.MatmulPerfMode.DoubleRow

    nc = bacc.Bacc("TRN2", target_bir_lowering=False, debug=False,
                   num_devices=NCORES)

    f32r = mybir.dt.float32r
    if scheme == "f32r":
        xh_d = nc.dram_tensor("xh", [NT, P, KT, 512], f32r,
                              kind="ExternalInput")
        wh_d = nc.dram_tensor("wh", [GJ, P, 3, KT, P], f32r,
                              kind="ExternalInput")
    elif scheme == "bf16x3":
        xh_d = nc.dram_tensor("xh", [P, KT, TB], bf16, kind="ExternalInput")
        xl_d = nc.dram_tensor("xl", [P, KT, TB], bf16, kind="ExternalInput")
        wh_d = nc.dram_tensor("wh", [GJ, P, 3, KT, P], bf16,
                              kind="ExternalInput")
        wl_d = nc.dram_tensor("wl", [GJ, P, 3, KT, P], bf16,
                              kind="ExternalInput")
    else:
        xh_d = nc.dram_tensor("xh", [P, KT, TB], f16, kind="ExternalInput")
        x8_d = nc.dram_tensor("x8", [P, KT, 2, TB], f8, kind="ExternalInput")
        wh_d = nc.dram_tensor("wh", [GJ, P, 3, KT, P], f16,
                              kind="ExternalInput")
        w8_d = nc.dram_tensor("w8", [GJ, P, 2, KT, 2, P], f8,
                              kind="ExternalInput")
    br_d, bz_d, bin_d, bhn_d, out_d = _common_io(nc, mybir, KT, GJ, TB,
                                                 scheme)

    with tile.TileContext(nc) as tc:
        with (
            tc.tile_pool(name="xp", bufs=1) as xp,
            tc.tile_pool(name="wp", bufs=2) as wp,
            tc.tile_pool(name="gp", bufs=2) as gp,
            tc.tile_pool(name="pp", bufs=7, space="PSUM") as pp,
        ):
            bp, lp, op = xp, gp, gp
            # X arrives on the ACT HWDGE ring in k-chunks so the first
            # matmuls (and the W loads on the sync ring) aren't stuck
            # behind one monolithic 8MB transfer.
            XC = 4 if KT % 4 == 0 else 1
            if scheme == "f32r":
                # X on the ACT HWDGE ring, n-half-major: the first
                # n-tile's X (4MB) lands in half the time, so j0/n0
                # matmuls start earlier while W streams on the sync
                # ring.  Graded chunks within each half.
                xh_sb = xp.tile([P, KT, TB], f32r, tag="xh")
                x2_sb = None
                br_sb = bp.tile([P, GJ], f32, tag="br")
                nc.gpsimd.dma_start(out=br_sb[:], in_=br_d[:])
                bz_sb = bp.tile([P, GJ], f32, tag="bz")
                nc.gpsimd.dma_start(out=bz_sb[:], in_=bz_d[:])
                bin_sb = bp.tile([P, GJ], f32, tag="bin")
                nc.gpsimd.dma_start(out=bin_sb[:], in_=bin_d[:])
                bhn_sb = bp.tile([P, GJ], f32, tag="bhn")
                nc.gpsimd.dma_start(out=bhn_sb[:], in_=bhn_d[:])
                bounds = [0, 1, 2, 4, 8, KT] if KT == 16 else \
                    list(range(0, KT + 1, XC))
                for n in range(NT):
                    ns = slice(n * 512, (n + 1) * 512)
                    for a, b in zip(bounds[:-1], bounds[1:]):
                        nc.scalar.dma_start(out=xh_sb[:, a:b, ns],
                                            in_=xh_d[n, :, a:b])
            elif scheme == "bf16x3":
                xh_sb = xp.tile([P, KT, TB], bf16, tag="xh")
                x2_sb = xp.tile([P, KT, TB], bf16, tag="x2")
                for c in range(0, KT, XC):
                    cs = slice(c, c + XC)
                    nc.scalar.dma_start(out=xh_sb[:, cs], in_=xh_d[:, cs])
                    nc.scalar.dma_start(out=x2_sb[:, cs], in_=xl_d[:, cs])
            else:
                # X on the ACT HWDGE ring in graded chunks (small first so
                # the k=0 matmuls can start early), W on the sync ring.
                xh_sb = xp.tile([P, KT, TB], f16, tag="xh")
                x2_sb = xp.tile([P, KT, 2, TB], f8, tag="x2")
                bounds = [0, 1, 2, 4, 8, KT] if KT == 16 else \
                    list(range(0, KT + 1, XC))
                for a, b in zip(bounds[:-1], bounds[1:]):
                    cs = slice(a, b)
                    nc.scalar.dma_start(out=xh_sb[:, cs], in_=xh_d[:, cs])
                    nc.scalar.dma_start(out=x2_sb[:, cs], in_=x8_d[:, cs])

            # Warm the PE (HAM un-throttle needs ~3.4us of sustained matmul
            # activity) while the input DMAs land: dummy matmuls on a
            # memset tile into a spare PSUM bank.
            warm = bp.tile([P, 512], f16 if scheme != "bf16x3" else bf16,
                           tag="warm")
            nc.vector.memset(warm[:], 0)
            wps = pp.tile([P, 512], f32, tag="warmps", name="warmps",
                          bufs=1)
            NWARM = 32 if scheme == "f32r" else 24
            for r_ in range(NWARM):
                nc.tensor.matmul(wps[:, 0:256], warm[:, 0:P], warm[:, 0:256],
                                 start=(r_ == 0), stop=(r_ == NWARM - 1),
                                 skip_group_check=True)

            if scheme != "f32r":
                br_sb = bp.tile([P, GJ], f32, tag="br")
                nc.gpsimd.dma_start(out=br_sb[:], in_=br_d[:])
                bz_sb = bp.tile([P, GJ], f32, tag="bz")
                nc.gpsimd.dma_start(out=bz_sb[:], in_=bz_d[:])
                bin_sb = bp.tile([P, GJ], f32, tag="bin")
                nc.gpsimd.dma_start(out=bin_sb[:], in_=bin_d[:])
                bhn_sb = bp.tile([P, GJ], f32, tag="bhn")
                nc.gpsimd.dma_start(out=bhn_sb[:], in_=bhn_d[:])

            for j in range(GJ):
                if scheme == "f32r":
                    wh_sb = wp.tile([P, 3, KT, P], f32r, tag="wh")
                    for g in range(3):
                        nc.sync.dma_start(out=wh_sb[:, g],
                                          in_=wh_d[j, :, g])
                elif scheme == "bf16x3":
                    wh_sb = wp.tile([P, 3, KT, P], bf16, tag="wh")
                    nc.sync.dma_start(out=wh_sb[:], in_=wh_d[j])
                    w2_sb = wp.tile([P, 3, KT, P], bf16, tag="w2")
                    nc.sync.dma_start(out=w2_sb[:], in_=wl_d[j])
                else:
                    wh_sb = wp.tile([P, 3, KT, P], f16, tag="wh")
                    nc.sync.dma_start(out=wh_sb[:], in_=wh_d[j])
                    w2_sb = wp.tile([P, 2, KT, 2, P], f8, tag="w2")
                    nc.sync.dma_start(out=w2_sb[:], in_=w8_d[j])

                if scheme == "f32r":
                    # Single fp32r pass per gate: ~fp16 speed, 2x better
                    # accuracy, fp32 data straight from HBM.  g-outer so
                    # gate g only waits on its own W chunk.  Columns are
                    # b-major (t inner) so each 512-col n-tile is
                    # LIF-complete and ships its output immediately.
                    BN = TB // 4 // NT
                    cur = gp.tile([P, NT, BN, 4], f32, tag="cur")
                    out_sb = op.tile([P, NT, 3, BN], f32, tag="out")
                    nsl = [slice(n * 512, (n + 1) * 512) for n in range(NT)]
                    ps6 = [[pp.tile([P, 512], f32, tag="ps",
                                    name=f"ps_{j}_{g}_{n}")
                            for n in range(NT)] for g in range(3)]
                    for n in range(NT):
                        for g in range(3):
                            for k in range(KT):
                                nc.tensor.matmul(
                                    ps6[g][n][:], wh_sb[:, g, k, :],
                                    xh_sb[:, k, nsl[n]],
                                    start=(k == 0), stop=(k == KT - 1),
                                    skip_group_check=True)
                    for n in range(NT):
                        ns = nsl[n]
                        ps3 = [ps6[0][n], ps6[1][n], ps6[2][n]]
                        bj = lambda t: t[:, j:j + 1]
                        r = gp.tile([P, 512], f32, tag="r")
                        zb = gp.tile([P, 512], f32, tag="zb")
                        rbn = gp.tile([P, 512], f32, tag="rbn")
                        n2 = gp.tile([P, 512], f32, tag="n2")
                        a01 = lp.tile([P, BN], f32, tag="a01")
                        a012 = lp.tile([P, BN], f32, tag="a012")
                        nsplit = 1
                        for h in range(nsplit):
                            w = 512 // nsplit
                            cs = slice(h * w, (h + 1) * w)
                            bs = slice(h * (BN // nsplit),
                                       (h + 1) * (BN // nsplit))
                            nc.vector.tensor_scalar(r[:, cs], ps3[0][:, cs],
                                                    bj(br_sb), 0.0,
                                                    A.add, A.is_gt)
                            nc.vector.tensor_scalar(zb[:, cs], ps3[1][:, cs],
                                                    bj(bz_sb), 0.0,
                                                    A.add, A.is_le)
                            # rbn = r*b_hn + b_in (ready before y_n stops)
                            nc.vector.tensor_scalar(rbn[:, cs], r[:, cs],
                                                    bj(bhn_sb), bj(bin_sb),
                                                    A.mult, A.add)
                            nc.vector.scalar_tensor_tensor(
                                n2[:, cs], ps3[2][:, cs], 1.0, rbn[:, cs],
                                A.mult, A.add)
                            nc.vector.scalar_tensor_tensor(
                                cur[:, n, bs], n2[:, cs], 0.0, zb[:, cs],
                                A.is_gt, A.mult)
                            # LIF collapses to booleans (cur in {0,1},
                            # threshold 1, beta 0.99):
                            #   s1 = c0*c1
                            #   s2 = c2*[c0+c1 > 0]
                            #   s3 = c3*[c0+c1+c2 > 0]
                            c0 = cur[:, n, bs, 0]
                            c1 = cur[:, n, bs, 1]
                            c2 = cur[:, n, bs, 2]
                            c3 = cur[:, n, bs, 3]
                            s1 = out_sb[:, n, 0, bs]
                            s2 = out_sb[:, n, 1, bs]
                            s3 = out_sb[:, n, 2, bs]
                            nc.vector.tensor_tensor(a01[:, bs], c0, c1,
                                                    A.add)
                            nc.vector.tensor_tensor(s1, c0, c1, A.mult)
                            nc.vector.scalar_tensor_tensor(
                                s2, a01[:, bs], 0.0, c2, A.is_gt, A.mult)
                            nc.vector.tensor_tensor(a012[:, bs], a01[:, bs],
                                                    c2, A.add)
                            nc.vector.scalar_tensor_tensor(
                                s3, a012[:, bs], 0.0, c3, A.is_gt, A.mult)
                            if nsplit == 1:
                                nc.gpsimd.dma_start(out=out_d[j, n],
                                                    in_=out_sb[:, n])
                            else:
                                nc.gpsimd.dma_start(
                                    out=out_d[j, n, :, :, bs],
                                    in_=out_sb[:, n, :, bs])
                    continue

                cur = gp.tile([P, TB], f32, tag="cur")
                nsl = [slice(n * 512, (n + 1) * 512) for n in range(NT)]
                if scheme == "bf16x3":
                    psg = [[pp.tile([P, 512], f32, tag="ps",
                                     name=f"ps_{j}_{g}_{n}")
                            for n in range(NT)] for g in range(3)]
                    for g in range(3):
                        for n in range(NT):
                            pst = psg[g][n]
                            ns = nsl[n]
                            for k in range(KT):
                                xh_k = xh_sb[:, k, ns]
                                xl_k = x2_sb[:, k, ns]
                                nc.tensor.matmul(pst[:], wh_sb[:, g, k, :],
                                                 xh_k, start=(k == 0),
                                                 stop=False)
                                nc.tensor.matmul(pst[:], wh_sb[:, g, k, :],
                                                 xl_k, start=False, stop=False)
                                nc.tensor.matmul(pst[:], w2_sb[:, g, k, :],
                                                 xh_k, start=False,
                                                 stop=(k == KT - 1))
                else:
                    # Alternate fp16 MMs with fp8-DR MMs across the 3 PSUM
                    # banks of one n-tile so every 256-col DR weight-load
                    # hides under a preceding fp16 MM.  One n-tile at a
                    # time: its gate DVE work starts while the next n-tile
                    # (or j-group) is still on the PE.
                    # g=0 (r-gate) skips the fp8 correction: an r flip only
                    # matters when y_n lands inside the +-b_hn window
                    # (P ~ 0.8%), so fp16-main accuracy is plenty for r.
                    psg = [[None] * NT for _ in range(3)]
                    for n in range(NT):
                        for g in range(3):
                            psg[g][n] = pp.tile([P, 512], f32, tag="ps",
                                                name=f"ps_{j}_{g}_{n}")
                        for k in range(KT):
                            for g in range(3):
                                w16 = wh_sb[:, g, k, :]
                                nc.tensor.matmul(psg[g][n][:], w16,
                                                 xh_sb[:, k, nsl[n]],
                                                 start=(k == 0),
                                                 stop=(g == 0 and
                                                       k == KT - 1),
                                                 skip_group_check=True)
                                if g != 0:
                                    w8k = w2_sb[:, g - 1, k, :, :]
                                    nc.tensor.matmul(psg[g][n][:], w8k,
                                                     x2_sb[:, k, :, nsl[n]],
                                                     perf_mode=DR,
                                                     start=False,
                                                     stop=(k == KT - 1),
                                                     skip_group_check=True)

                for n in range(NT):
                    ns = nsl[n]
                    ps = [psg[0][n], psg[1][n], psg[2][n]]
                    bj = lambda t: t[:, j:j + 1]
                    r = gp.tile([P, 512], f32, tag="r")
                    zb = gp.tile([P, 512], f32, tag="zb")
                    if scheme in ("bf16x3", "f32r"):
                        # r = (y_r + b_r) > 0 ; zbar = (y_z + b_z) <= 0
                        nc.vector.tensor_scalar(r[:], ps[0][:], bj(br_sb), 0.0,
                                                A.add, A.is_gt)
                        nc.vector.tensor_scalar(zb[:], ps[1][:], bj(bz_sb),
                                                0.0, A.add, A.is_le)
                        # nn = y_n + b_in (ACT engine, matches ref rounding)
                        nn = gp.tile([P, 512], f32, tag="nn")
                        nc.scalar.activation(
                            nn[:], ps[2][:],
                            mybir.ActivationFunctionType.Identity,
                            bias=bj(bin_sb), scale=1.0)
                        # n2 = r*b_hn + nn ;  cur = (n2 > 0) * zbar
                        n2 = gp.tile([P, 512], f32, tag="n2")
                        nc.vector.scalar_tensor_tensor(n2[:], r[:], bj(bhn_sb),
                                                       nn[:], A.mult, A.add)
                        nc.vector.scalar_tensor_tensor(cur[:, ns], n2[:], 0.0,
                                                       zb[:], A.is_gt, A.mult)
                    else:
                        # psum holds y*2^16; br/bz arrive pre-scaled by
                        # -2^16 so the compare absorbs bias and scale.
                        nc.vector.tensor_scalar(r[:], ps[0][:], bj(br_sb),
                                                None, A.is_gt)
                        nc.vector.tensor_scalar(zb[:], ps[1][:], bj(bz_sb),
                                                None, A.is_le)
                        # rbn = r*b_hn + b_in ; n2 = y_n*2^-16 + rbn
                        rbn = gp.tile([P, 512], f32, tag="rbn")
                        nc.vector.tensor_scalar(rbn[:], r[:], bj(bhn_sb),
                                                bj(bin_sb), A.mult, A.add)
                        n2 = gp.tile([P, 512], f32, tag="n2")
                        nc.vector.scalar_tensor_tensor(n2[:], ps[2][:],
                                                       1.0 / SCALE, rbn[:],
                                                       A.mult, A.add)
                        nc.vector.scalar_tensor_tensor(cur[:, ns], n2[:], 0.0,
                                                       zb[:], A.is_gt, A.mult)

                # LIF over the 4 timesteps (t-major layout in cur)
                out_sb = op.tile([P, 3 * BT], f32, tag="out")
                c0 = cur[:, 0 * BT:1 * BT]
                c1 = cur[:, 1 * BT:2 * BT]
                c2 = cur[:, 2 * BT:3 * BT]
                c3 = cur[:, 3 * BT:4 * BT]
                s1 = out_sb[:, 0 * BT:1 * BT]
                s2 = out_sb[:, 1 * BT:2 * BT]
                s3 = out_sb[:, 2 * BT:3 * BT]

                m2 = lp.tile([P, BT], f32, tag="m2")
                nc.vector.scalar_tensor_tensor(m2[:], c0, 0.99, c1,
                                               A.mult, A.add)
                nc.vector.tensor_scalar(s1, m2[:], 1.0, None, A.is_gt)
                t2 = lp.tile([P, BT], f32, tag="t2")
                nc.vector.tensor_tensor(t2[:], c2, s1, A.subtract)
                m3 = lp.tile([P, BT], f32, tag="m3")
                nc.vector.scalar_tensor_tensor(m3[:], m2[:], 0.99, t2[:],
                                               A.mult, A.add)
                nc.vector.tensor_scalar(s2, m3[:], 1.0, None, A.is_gt)
                t3 = lp.tile([P, BT], f32, tag="t3")
                nc.vector.tensor_tensor(t3[:], c3, s2, A.subtract)
                m4 = lp.tile([P, BT], f32, tag="m4")
                nc.vector.scalar_tensor_tensor(m4[:], m3[:], 0.99, t3[:],
                                               A.mult, A.add)
                nc.vector.tensor_scalar(s3, m4[:], 1.0, None, A.is_gt)

                nc.gpsimd.dma_start(out=out_d[j], in_=out_sb[:])

    nc.compile()
    return nc


def _blocked_w(Wt, KT, GJ):
    """[I, 3H] -> (j, p, g, k, m) blocked layout."""
    Wb = Wt.reshape(KT, P, 3, GJ, P).transpose(3, 1, 2, 0, 4)
    return np.ascontiguousarray(Wb)


def prep_weights(W_ih, b_ih, b_hh, KT, GJ, scheme=None):
    """Host-side packing of weights/biases (shared across cores)."""
    scheme = scheme or SCHEME
    threeH = 3 * GJ * P
    II = KT * P
    Wt = np.ascontiguousarray(W_ih[:threeH, :II].T)          # [I, 3H] fp32

    HH = GJ * P
    b_r = (b_ih[0:HH] + b_hh[0:HH]).astype(np.float32)
    b_z = (b_ih[HH:2 * HH] + b_hh[HH:2 * HH]).astype(np.float32)
    b_in = b_ih[2 * HH:3 * HH].astype(np.float32)
    b_hn = b_hh[2 * HH:3 * HH].astype(np.float32)
    asb = lambda b: np.ascontiguousarray(b.reshape(GJ, P).T)

    if scheme == "f32r":
        Wb = _blocked_w(Wt, KT, GJ)
        return {"wh": Wb, "br": asb(b_r), "bz": asb(b_z),
                "bin": asb(b_in), "bhn": asb(b_hn)}

    if scheme == "bf16x3":
        Wb = _blocked_w(Wt, KT, GJ)
        wh = Wb.astype(BF16)
        wl = (Wb - wh.astype(np.float32)).astype(BF16)
        return {"wh": wh, "wl": wl, "br": asb(b_r), "bz": asb(b_z),
                "bin": asb(b_in), "bhn": asb(b_hn)}

    Wb = _blocked_w(Wt, KT, GJ)                              # [GJ,P,3,KT,P]
    wh16 = Wb.astype(np.float16)
    wl = Wb - wh16.astype(np.float32)
    wh_scaled = (wh16.astype(np.float32) * SW_H).astype(np.float16)
    # fp8 correction factors only for the z/n gates (g=1,2)
    w8 = np.empty((Wb.shape[0], P, 2, KT, 2, P), dtype=FP8)
    w8[:, :, :, :, 0, :] = (wh16[:, :, 1:3].astype(np.float32)
                            * SW8_H).astype(FP8)
    w8[:, :, :, :, 1, :] = (wl[:, :, 1:3] * SW8_L).astype(FP8)
    return {"wh": wh_scaled, "w8": w8,
            "br": asb(-b_r * SCALE), "bz": asb(-b_z * SCALE),
            "bin": asb(b_in), "bhn": asb(b_hn)}


def prep_x(x_core, KT, BT, scheme=None):
    """x_core: [BL, I, T] fp32 -> per-core input dict."""
    scheme = scheme or SCHEME
    II = KT * P
    if scheme == "f32r":
        # b-major columns (t inner): col = b*4 + t; n-half-major DRAM
        xt = x_core[:, :II, :].transpose(1, 0, 2)      # [I, BL, T]
        xt = xt.reshape(KT, P, 4 * BT).transpose(1, 0, 2)  # [P, KT, TB]
        NT = (4 * BT) // 512
        xt = xt.reshape(P, KT, NT, 512).transpose(2, 0, 1, 3)
        return {"xh": np.ascontiguousarray(xt)}
    xt = x_core[:, :II, :].transpose(1, 2, 0)          # [I, T, BL]
    xt = xt.reshape(KT, P, 4 * BT).transpose(1, 0, 2)  # [P, KT, TB]
    xt = np.ascontiguousarray(xt)
    if scheme == "bf16x3":
        xh = xt.astype(BF16)
        xl = (xt - xh.astype(np.float32)).astype(BF16)
        return {"xh": xh, "xl": xl}
    xh16 = xt.astype(np.float16)
    xl = xt - xh16.astype(np.float32)
    xh_scaled = (xh16.astype(np.float32) * SX_H).astype(np.float16)
    x8 = np.empty((P, KT, 2, 4 * BT), dtype=FP8)
    x8[:, :, 0, :] = (xl * SX8_L).astype(FP8)
    x8[:, :, 1, :] = xh16.astype(np.float32).astype(FP8)
    return {"xh": xh_scaled, "x8": x8}


def unpack_out(out, GJ, BT):
    """out: [GJ, P, 3*BT] fp32 -> spikes [BL, H', 4] with t=0 zeros."""
    HH = GJ * P
    arr = out.reshape(HH, 3, BT)                     # [h, t-1, b]
    res = np.zeros((BT, HH, 4), dtype=np.float32)
    res[:, :, 1:4] = arr.transpose(2, 0, 1)
    return res


def unpack_out_f32r(out, GJ, BT):
    """out: [GJ, NT, P, 3, BN] -> spikes [BL, H', 4] with t=0 zeros."""
    HH = GJ * P
    NT, BN = out.shape[1], out.shape[4]
    arr = out.transpose(1, 4, 0, 2, 3).reshape(NT * BN, HH, 3)
    res = np.zeros((NT * BN, HH, 4), dtype=np.float32)
    res[:, :, 1:4] = arr
    return res


def kernel(inputs, W_ih, b_ih, W_hh, b_hh):
    from concourse.bass_utils import run_bass_kernel_spmd

    # BT = batch rows per timestep per core (= local batch size BL)
    KT, GJ, BT = I // P, H // P, B // NCORES
    key = (KT, GJ, BT, SCHEME)
    if key not in _CACHE:
        _CACHE[key] = build_nc(KT, GJ, BT)
    nc = _CACHE[key]

    wmap = prep_weights(np.asarray(W_ih, dtype=np.float32),
                        np.asarray(b_ih, dtype=np.float32),
                        np.asarray(b_hh, dtype=np.float32), KT, GJ)

    x = np.asarray(inputs, dtype=np.float32)
    in_maps = []
    BL = B // NCORES
    for c in range(NCORES):
        m = dict(wmap)
        m.update(prep_x(x[c * BL:(c + 1) * BL], KT, BT))
        in_maps.append(m)

    res = run_bass_kernel_spmd(nc, in_maps, list(range(NCORES)), trace=TRACE)
    global LAST_EXEC_NS, LAST_RESULTS
    LAST_EXEC_NS = res.exec_time_ns
    LAST_RESULTS = res

    unpack = unpack_out_f32r if SCHEME == "f32r" else unpack_out
    out = np.empty((B, H, T), dtype=np.float32)
    for c in range(NCORES):
        out[c * BL:(c + 1) * BL] = unpack(res.results[c]["out"], GJ, BT)
    return out



# revision 50
# speedup vs baseline: 1.0121x; 1.0121x over previous
"""Trainium2 Bass kernel for nn_GRUCell_21612275433682.

Math (from the reference):
  - h0 = 0, so the W_hh matmul is dead: only b_hh enters the gates.
  - y = x @ W_ih.T            (the single big GEMM, [B*T, I] @ [I, 3H])
  - r = (y_r + b_ih_r + b_hh_r > 0)
  - z = (y_z + b_ih_z + b_hh_z > 0)
  - n = (y_n + b_ih_n + r*b_hh_n > 0)
  - cur = (1-z)*n   in {0,1}
  - LIF over T=4 steps:  mem' = 0.99*mem + cur_t - spk_{t-1};  spk_t = (mem' > 1)
    spk_0 is identically 0 (mem1 = cur0 <= 1).

Strategy: pure data parallel over 8 cores (B sharded 256/core). Per core one
[3H=6144, TB=1024] x [I=2048] GEMM with W stationary ([I,3H] tiles) and X
moving. Output layout [3h partitions, b-major (b*4+t) free] so biases are
per-partition scalars and the LIF is pure free-dim slicing.

GEMM precision schemes (SCHEME):
  - "f32r":    single fp32r ("replicated fp32") pass per gate.  Measured on
               HW: ~227ns per [128x128]x[128x512] tile (vs fp16's 216) with
               ~2x lower error than fp16 (y-rms 8.7e-5 vs 1.7e-4 at K=2048)
               and fp32 data straight from HBM (no host-side splitting).
               The PE quantizes each operand to ~11-12 mantissa bits
               internally (verified: 11-bit values pass through exactly;
               host pre-rounding cannot reduce the total error).
  - "bf16x3":  W,X split into bf16 hi/lo; 3 bf16 passes.
  - "f16f8":   fp16 hi pass + both cross terms packed into one fp8e4m3
               DoubleRow pass (measured: the DR pass costs a FULL fp16-pass
               equivalent, ~230ns, not the 0.5x the cost model claims).

The LIF scan collapses to booleans (cur in {0,1}, threshold 1, beta 0.99,
and one decay step never drops a positive residue below the spike gap):
  s1 = c0*c1, s2 = c2*[c0+c1>0], s3 = c3*[c0+c1+c2>0]; s0 identically 0.
This is exact vs the reference fp32 scan (the only boundary case, mem==1.0,
is exact in fp32 and the reference compares strictly).
"""

import numpy as np
import ml_dtypes

BF16 = ml_dtypes.bfloat16
FP8 = ml_dtypes.float8_e4m3

# Full problem sizes (hardcoded per contract)
B, I, H, T = 2048, 2048, 2048, 4
NCORES = 8
P = 128

SCHEME = "f32r"

# scheme f16f8 scale choices (powers of two; see product-scale table below)
#   main:  (wh * 2^8) @ (xh * 2^8)            -> y_main * 2^16
#   cross: fp8(wh*2^5) @ fp8(xl*2^11)         -> cross1 * 2^16
#          fp8(wl*2^16) @ fp8(xh)             -> cross2 * 2^16
SW_H, SX_H = 256.0, 256.0
SW8_H, SX8_L = 32.0, 2048.0
SW8_L, SX8_H = 65536.0, 1.0
SCALE = 65536.0

_CACHE = {}

# test-harness knobs (grading path leaves these alone)
TRACE = False
LAST_EXEC_NS = None
LAST_RESULTS = None


def _common_io(nc, mybir, KT, GJ, TB, scheme):
    f32 = mybir.dt.float32
    br_d = nc.dram_tensor("br", [P, GJ], f32, kind="ExternalInput")
    bz_d = nc.dram_tensor("bz", [P, GJ], f32, kind="ExternalInput")
    bin_d = nc.dram_tensor("bin", [P, GJ], f32, kind="ExternalInput")
    bhn_d = nc.dram_tensor("bhn", [P, GJ], f32, kind="ExternalInput")
    if scheme == "f32r":
        # b-major columns: out[j, n, p, t, bb]
        NT = TB // 512
        BN = TB // 4 // NT
        out_d = nc.dram_tensor("out", [GJ, NT, P, 3, BN], f32,
                               kind="ExternalOutput")
    else:
        out_d = nc.dram_tensor("out", [GJ, P, 3 * (TB // 4)], f32,
                               kind="ExternalOutput")
    return br_d, bz_d, bin_d, bhn_d, out_d


def build_nc(KT, GJ, BT, scheme=None):
    """Build the per-core Bass program.

    KT: number of 128-wide K tiles (I = 128*KT)
    GJ: number of 128-row h-tile groups per gate (H = 128*GJ)
    BT: batch rows per timestep per core (TB = 4*BT total moving columns)
    """
    import concourse.mybir as mybir
    import concourse.tile as tile
    from concourse import bacc

    scheme = scheme or SCHEME
    TB = 4 * BT
    NT = TB // 512
    assert NT * 512 == TB

    f32 = mybir.dt.float32
    bf16 = mybir.dt.bfloat16
    f16 = mybir.dt.float16
    f8 = mybir.dt.float8e4
    A = mybir.AluOpType
    DR = mybir.MatmulPerfMode.DoubleRow

    nc = bacc.Bacc("TRN2", target_bir_lowering=False, debug=False,
                   num_devices=NCORES)

    f32r = mybir.dt.float32r
    if scheme == "f32r":
        xh_d = nc.dram_tensor("xh", [NT, P, KT, 512], f32r,
                              kind="ExternalInput")
        wh_d = nc.dram_tensor("wh", [GJ, P, 3, KT, P], f32r,
                              kind="ExternalInput")
    elif scheme == "bf16x3":
        xh_d = nc.dram_tensor("xh", [P, KT, TB], bf16, kind="ExternalInput")
        xl_d = nc.dram_tensor("xl", [P, KT, TB], bf16, kind="ExternalInput")
        wh_d = nc.dram_tensor("wh", [GJ, P, 3, KT, P], bf16,
                              kind="ExternalInput")
        wl_d = nc.dram_tensor("wl", [GJ, P, 3, KT, P], bf16,
                              kind="ExternalInput")
    else:
        xh_d = nc.dram_tensor("xh", [P, KT, TB], f16, kind="ExternalInput")
        x8_d = nc.dram_tensor("x8", [P, KT, 2, TB], f8, kind="ExternalInput")
        wh_d = nc.dram_tensor("wh", [GJ, P, 3, KT, P], f16,
                              kind="ExternalInput")
        w8_d = nc.dram_tensor("w8", [GJ, P, 2, KT, 2, P], f8,
                              kind="ExternalInput")
    br_d, bz_d, bin_d, bhn_d, out_d = _common_io(nc, mybir, KT, GJ, TB,
                                                 scheme)

    with tile.TileContext(nc) as tc:
        with (
            tc.tile_pool(name="xp", bufs=1) as xp,
            tc.tile_pool(name="wp", bufs=2) as wp,
            tc.tile_pool(name="gp", bufs=2) as gp,
            tc.tile_pool(name="pp", bufs=7, space="PSUM") as pp,
        ):
            bp, lp, op = xp, gp, gp
            # X arrives on the ACT HWDGE ring in k-chunks so the first
            # matmuls (and the W loads on the sync ring) aren't stuck
            # behind one monolithic 8MB transfer.
            XC = 4 if KT % 4 == 0 else 1
            if scheme == "f32r":
                # X on the ACT HWDGE ring, n-half-major: the first
                # n-tile's X (4MB) lands in half the time, so j0/n0
                # matmuls start earlier while W streams on the sync
                # ring.  Graded chunks within each half.
                xh_sb = xp.tile([P, KT, TB], f32r, tag="xh")
                x2_sb = None
                br_sb = bp.tile([P, GJ], f32, tag="br")
                nc.gpsimd.dma_start(out=br_sb[:], in_=br_d[:])
                bz_sb = bp.tile([P, GJ], f32, tag="bz")
                nc.gpsimd.dma_start(out=bz_sb[:], in_=bz_d[:])
                bin_sb = bp.tile([P, GJ], f32, tag="bin")
                nc.gpsimd.dma_start(out=bin_sb[:], in_=bin_d[:])
                bhn_sb = bp.tile([P, GJ], f32, tag="bhn")
                nc.gpsimd.dma_start(out=bhn_sb[:], in_=bhn_d[:])
                bounds = [0, 1, 2, 4, 8, KT] if KT == 16 else \
                    list(range(0, KT + 1, XC))
                for n in range(NT):
                    ns = slice(n * 512, (n + 1) * 512)
                    for a, b in zip(bounds[:-1], bounds[1:]):
                        nc.scalar.dma_start(out=xh_sb[:, a:b, ns],
                                            in_=xh_d[n, :, a:b])
            elif scheme == "bf16x3":
                xh_sb = xp.tile([P, KT, TB], bf16, tag="xh")
                x2_sb = xp.tile([P, KT, TB], bf16, tag="x2")
                for c in range(0, KT, XC):
                    cs = slice(c, c + XC)
                    nc.scalar.dma_start(out=xh_sb[:, cs], in_=xh_d[:, cs])
                    nc.scalar.dma_start(out=x2_sb[:, cs], in_=xl_d[:, cs])
            else:
                # X on the ACT HWDGE ring in graded chunks (small first so
                # the k=0 matmuls can start early), W on the sync ring.
                xh_sb = xp.tile([P, KT, TB], f16, tag="xh")
                x2_sb = xp.tile([P, KT, 2, TB], f8, tag="x2")
                bounds = [0, 1, 2, 4, 8, KT] if KT == 16 else \
                    list(range(0, KT + 1, XC))
                for a, b in zip(bounds[:-1], bounds[1:]):
                    cs = slice(a, b)
                    nc.scalar.dma_start(out=xh_sb[:, cs], in_=xh_d[:, cs])
                    nc.scalar.dma_start(out=x2_sb[:, cs], in_=x8_d[:, cs])

            # Warm the PE (HAM un-throttle needs ~3.4us of sustained matmul
            # activity) while the input DMAs land: dummy matmuls on a
            # memset tile into a spare PSUM bank.
            warm = bp.tile([P, 512], f16 if scheme != "bf16x3" else bf16,
                           tag="warm")
            nc.vector.memset(warm[:], 0)
            wps = pp.tile([P, 512], f32, tag="warmps", name="warmps",
                          bufs=1)
            NWARM = 32 if scheme == "f32r" else 24
            for r_ in range(NWARM):
                nc.tensor.matmul(wps[:, 0:256], warm[:, 0:P], warm[:, 0:256],
                                 start=(r_ == 0), stop=(r_ == NWARM - 1),
                                 skip_group_check=True)

            if scheme != "f32r":
                br_sb = bp.tile([P, GJ], f32, tag="br")
                nc.gpsimd.dma_start(out=br_sb[:], in_=br_d[:])
                bz_sb = bp.tile([P, GJ], f32, tag="bz")
                nc.gpsimd.dma_start(out=bz_sb[:], in_=bz_d[:])
                bin_sb = bp.tile([P, GJ], f32, tag="bin")
                nc.gpsimd.dma_start(out=bin_sb[:], in_=bin_d[:])
                bhn_sb = bp.tile([P, GJ], f32, tag="bhn")
                nc.gpsimd.dma_start(out=bhn_sb[:], in_=bhn_d[:])

            for j in range(GJ):
                if scheme == "f32r":
                    wh_sb = wp.tile([P, 3, KT, P], f32r, tag="wh")
                    for g in range(3):
                        nc.sync.dma_start(out=wh_sb[:, g],
                                          in_=wh_d[j, :, g])
                elif scheme == "bf16x3":
                    wh_sb = wp.tile([P, 3, KT, P], bf16, tag="wh")
                    nc.sync.dma_start(out=wh_sb[:], in_=wh_d[j])
                    w2_sb = wp.tile([P, 3, KT, P], bf16, tag="w2")
                    nc.sync.dma_start(out=w2_sb[:], in_=wl_d[j])
                else:
                    wh_sb = wp.tile([P, 3, KT, P], f16, tag="wh")
                    nc.sync.dma_start(out=wh_sb[:], in_=wh_d[j])
                    w2_sb = wp.tile([P, 2, KT, 2, P], f8, tag="w2")
                    nc.sync.dma_start(out=w2_sb[:], in_=w8_d[j])

                if scheme == "f32r":
                    # Single fp32r pass per gate: ~fp16 speed, 2x better
                    # accuracy, fp32 data straight from HBM.  g-outer so
                    # gate g only waits on its own W chunk.  Columns are
                    # b-major (t inner) so each 512-col n-tile is
                    # LIF-complete and ships its output immediately.
                    BN = TB // 4 // NT
                    cur = gp.tile([P, NT, BN, 4], f32, tag="cur")
                    out_sb = op.tile([P, NT, 3, BN], f32, tag="out")
                    nsl = [slice(n * 512, (n + 1) * 512) for n in range(NT)]
                    ps6 = [[pp.tile([P, 512], f32, tag="ps",
                                    name=f"ps_{j}_{g}_{n}")
                            for n in range(NT)] for g in range(3)]
                    for n in range(NT):
                        for g in range(3):
                            for k in range(KT):
                                nc.tensor.matmul(
                                    ps6[g][n][:], wh_sb[:, g, k, :],
                                    xh_sb[:, k, nsl[n]],
                                    start=(k == 0), stop=(k == KT - 1),
                                    skip_group_check=True)
                    for n in range(NT):
                        ns = nsl[n]
                        ps3 = [ps6[0][n], ps6[1][n], ps6[2][n]]
                        bj = lambda t: t[:, j:j + 1]
                        r = gp.tile([P, 512], f32, tag="r")
                        zb = gp.tile([P, 512], f32, tag="zb")
                        rbn = gp.tile([P, 512], f32, tag="rbn")
                        n2 = gp.tile([P, 512], f32, tag="n2")
                        a01 = lp.tile([P, BN], f32, tag="a01")
                        a012 = lp.tile([P, BN], f32, tag="a012")
                        nsplit = 1
                        for h in range(nsplit):
                            w = 512 // nsplit
                            cs = slice(h * w, (h + 1) * w)
                            bs = slice(h * (BN // nsplit),
                                       (h + 1) * (BN // nsplit))
                            nc.vector.tensor_scalar(r[:, cs], ps3[0][:, cs],
                                                    bj(br_sb), 0.0,
                                                    A.add, A.is_gt)
                            nc.vector.tensor_scalar(zb[:, cs], ps3[1][:, cs],
                                                    bj(bz_sb), 0.0,
                                                    A.add, A.is_le)
                            # rbn = r*b_hn + b_in (ready before y_n stops)
                            nc.vector.tensor_scalar(rbn[:, cs], r[:, cs],
                                                    bj(bhn_sb), bj(bin_sb),
                                                    A.mult, A.add)
                            nc.vector.scalar_tensor_tensor(
                                n2[:, cs], ps3[2][:, cs], 1.0, rbn[:, cs],
                                A.mult, A.add)
                            nc.vector.scalar_tensor_tensor(
                                cur[:, n, bs], n2[:, cs], 0.0, zb[:, cs],
                                A.is_gt, A.mult)
                            # LIF collapses to booleans (cur in {0,1},
                            # threshold 1, beta 0.99):
                            #   s1 = c0*c1
                            #   s2 = c2*[c0+c1 > 0]
                            #   s3 = c3*[c0+c1+c2 > 0]
                            c0 = cur[:, n, bs, 0]
                            c1 = cur[:, n, bs, 1]
                            c2 = cur[:, n, bs, 2]
                            c3 = cur[:, n, bs, 3]
                            s1 = out_sb[:, n, 0, bs]
                            s2 = out_sb[:, n, 1, bs]
                            s3 = out_sb[:, n, 2, bs]
                            nc.vector.tensor_tensor(a01[:, bs], c0, c1,
                                                    A.add)
                            nc.vector.tensor_tensor(s1, c0, c1, A.mult)
                            nc.vector.scalar_tensor_tensor(
                                s2, a01[:, bs], 0.0, c2, A.is_gt, A.mult)
                            nc.vector.tensor_tensor(a012[:, bs], a01[:, bs],
                                                    c2, A.add)
                            nc.vector.scalar_tensor_tensor(
                                s3, a012[:, bs], 0.0, c3, A.is_gt, A.mult)
                            if nsplit == 1:
                                nc.gpsimd.dma_start(out=out_d[j, n],
                                                    in_=out_sb[:, n])
                            else:
                                nc.gpsimd.dma_start(
                                    out=out_d[j, n, :, :, bs],
                                    in_=out_sb[:, n, :, bs])
                    continue

                cur = gp.tile([P, TB], f32, tag="cur")
                nsl = [slice(n * 512, (n + 1) * 512) for n in range(NT)]
                if scheme == "bf16x3":
                    psg = [[pp.tile([P, 512], f32, tag="ps",
                                     name=f"ps_{j}_{g}_{n}")
                            for n in range(NT)] for g in range(3)]
                    for g in range(3):
                        for n in range(NT):
                            pst = psg[g][n]
                            ns = nsl[n]
                            for k in range(KT):
                                xh_k = xh_sb[:, k, ns]
                                xl_k = x2_sb[:, k, ns]
                                nc.tensor.matmul(pst[:], wh_sb[:, g, k, :],
                                                 xh_k, start=(k == 0),
                                                 stop=False)
                                nc.tensor.matmul(pst[:], wh_sb[:, g, k, :],
                                                 xl_k, start=False, stop=False)
                                nc.tensor.matmul(pst[:], w2_sb[:, g, k, :],
                                                 xh_k, start=False,
                                                 stop=(k == KT - 1))
                else:
                    # Alternate fp16 MMs with fp8-DR MMs across the 3 PSUM
                    # banks of one n-tile so every 256-col DR weight-load
                    # hides under a preceding fp16 MM.  One n-tile at a
                    # time: its gate DVE work starts while the next n-tile
                    # (or j-group) is still on the PE.
                    # g=0 (r-gate) skips the fp8 correction: an r flip only
                    # matters when y_n lands inside the +-b_hn window
                    # (P ~ 0.8%), so fp16-main accuracy is plenty for r.
                    psg = [[None] * NT for _ in range(3)]
                    for n in range(NT):
                        for g in range(3):
                            psg[g][n] = pp.tile([P, 512], f32, tag="ps",
                                                name=f"ps_{j}_{g}_{n}")
                        for k in range(KT):
                            for g in range(3):
                                w16 = wh_sb[:, g, k, :]
                                nc.tensor.matmul(psg[g][n][:], w16,
                                                 xh_sb[:, k, nsl[n]],
                                                 start=(k == 0),
                                                 stop=(g == 0 and
                                                       k == KT - 1),
                                                 skip_group_check=True)
                                if g != 0:
                                    w8k = w2_sb[:, g - 1, k, :, :]
                                    nc.tensor.matmul(psg[g][n][:], w8k,
                                                     x2_sb[:, k, :, nsl[n]],
                                                     perf_mode=DR,
                                                     start=False,
                                                     stop=(k == KT - 1),
                                                     skip_group_check=True)

                for n in range(NT):
                    ns = nsl[n]
                    ps = [psg[0][n], psg[1][n], psg[2][n]]
                    bj = lambda t: t[:, j:j + 1]
                    r = gp.tile([P, 512], f32, tag="r")
                    zb = gp.tile([P, 512], f32, tag="zb")
                    if scheme in ("bf16x3", "f32r"):
                        # r = (y_r + b_r) > 0 ; zbar = (y_z + b_z) <= 0
                        nc.vector.tensor_scalar(r[:], ps[0][:], bj(br_sb), 0.0,
                                                A.add, A.is_gt)
                        nc.vector.tensor_scalar(zb[:], ps[1][:], bj(bz_sb),
                                                0.0, A.add, A.is_le)
                        # nn = y_n + b_in (ACT engine, matches ref rounding)
                        nn = gp.tile([P, 512], f32, tag="nn")
                        nc.scalar.activation(
                            nn[:], ps[2][:],
                            mybir.ActivationFunctionType.Identity,
                            bias=bj(bin_sb), scale=1.0)
                        # n2 = r*b_hn + nn ;  cur = (n2 > 0) * zbar
                        n2 = gp.tile([P, 512], f32, tag="n2")
                        nc.vector.scalar_tensor_tensor(n2[:], r[:], bj(bhn_sb),
                                                       nn[:], A.mult, A.add)
                        nc.vector.scalar_tensor_tensor(cur[:, ns], n2[:], 0.0,
                                                       zb[:], A.is_gt, A.mult)
                    else:
                        # psum holds y*2^16; br/bz arrive pre-scaled by
                        # -2^16 so the compare absorbs bias and scale.
                        nc.vector.tensor_scalar(r[:], ps[0][:], bj(br_sb),
                                                None, A.is_gt)
                        nc.vector.tensor_scalar(zb[:], ps[1][:], bj(bz_sb),
                                                None, A.is_le)
                        # rbn = r*b_hn + b_in ; n2 = y_n*2^-16 + rbn
                        rbn = gp.tile([P, 512], f32, tag="rbn")
                        nc.vector.tensor_scalar(rbn[:], r[:], bj(bhn_sb),
                                                bj(bin_sb), A.mult, A.add)
                        n2 = gp.tile([P, 512], f32, tag="n2")
                        nc.vector.scalar_tensor_tensor(n2[:], ps[2][:],
                                                       1.0 / SCALE, rbn[:],
                                                       A.mult, A.add)
                        nc.vector.scalar_tensor_tensor(cur[:, ns], n2[:], 0.0,
                                                       zb[:], A.is_gt, A.mult)

                # LIF over the 4 timesteps (t-major layout in cur)
                out_sb = op.tile([P, 3 * BT], f32, tag="out")
                c0 = cur[:, 0 * BT:1 * BT]
                c1 = cur[:, 1 * BT:2 * BT]
                c2 = cur[:, 2 * BT:3 * BT]
                c3 = cur[:, 3 * BT:4 * BT]
                s1 = out_sb[:, 0 * BT:1 * BT]
                s2 = out_sb[:, 1 * BT:2 * BT]
                s3 = out_sb[:, 2 * BT:3 * BT]

                m2 = lp.tile([P, BT], f32, tag="m2")
                nc.vector.scalar_tensor_tensor(m2[:], c0, 0.99, c1,
                                               A.mult, A.add)
                nc.vector.tensor_scalar(s1, m2[:], 1.0, None, A.is_gt)
                t2 = lp.tile([P, BT], f32, tag="t2")
                nc.vector.tensor_tensor(t2[:], c2, s1, A.subtract)
                m3 = lp.tile([P, BT], f32, tag="m3")
                nc.vector.scalar_tensor_tensor(m3[:], m2[:], 0.99, t2[:],
                                               A.mult, A.add)
                nc.vector.tensor_scalar(s2, m3[:], 1.0, None, A.is_gt)
                t3 = lp.tile([P, BT], f32, tag="t3")
                nc.vector.tensor_tensor(t3[:], c3, s2, A.subtract)
                m4 = lp.tile([P, BT], f32, tag="m4")
                nc.vector.scalar_tensor_tensor(m4[:], m3[:], 0.99, t3[:],
                                               A.mult, A.add)
                nc.vector.tensor_scalar(s3, m4[:], 1.0, None, A.is_gt)

                nc.gpsimd.dma_start(out=out_d[j], in_=out_sb[:])

    nc.compile()
    return nc


def _blocked_w(Wt, KT, GJ):
    """[I, 3H] -> (j, p, g, k, m) blocked layout."""
    Wb = Wt.reshape(KT, P, 3, GJ, P).transpose(3, 1, 2, 0, 4)
    return np.ascontiguousarray(Wb)


def prep_weights(W_ih, b_ih, b_hh, KT, GJ, scheme=None):
    """Host-side packing of weights/biases (shared across cores)."""
    scheme = scheme or SCHEME
    threeH = 3 * GJ * P
    II = KT * P
    Wt = np.ascontiguousarray(W_ih[:threeH, :II].T)          # [I, 3H] fp32

    HH = GJ * P
    b_r = (b_ih[0:HH] + b_hh[0:HH]).astype(np.float32)
    b_z = (b_ih[HH:2 * HH] + b_hh[HH:2 * HH]).astype(np.float32)
    b_in = b_ih[2 * HH:3 * HH].astype(np.float32)
    b_hn = b_hh[2 * HH:3 * HH].astype(np.float32)
    asb = lambda b: np.ascontiguousarray(b.reshape(GJ, P).T)

    if scheme == "f32r":
        Wb = _blocked_w(Wt, KT, GJ)
        return {"wh": Wb, "br": asb(b_r), "bz": asb(b_z),
                "bin": asb(b_in), "bhn": asb(b_hn)}

    if scheme == "bf16x3":
        Wb = _blocked_w(Wt, KT, GJ)
        wh = Wb.astype(BF16)
        wl = (Wb - wh.astype(np.float32)).astype(BF16)
        return {"wh": wh, "wl": wl, "br": asb(b_r), "bz": asb(b_z),
                "bin": asb(b_in), "bhn": asb(b_hn)}

    Wb = _blocked_w(Wt, KT, GJ)                              # [GJ,P,3,KT,P]
    wh16 = Wb.astype(np.float16)
    wl = Wb - wh16.astype(np.float32)
    wh_scaled = (wh16.astype(np.float32) * SW_H).astype(np.float16)
    # fp8 correction factors only for the z/n gates (g=1,2)
    w8 = np.empty((Wb.shape[0], P, 2, KT, 2, P), dtype=FP8)
    w8[:, :, :, :, 0, :] = (wh16[:, :, 1:3].astype(np.float32)
                            * SW8_H).astype(FP8)
    w8[:, :, :, :, 1, :] = (wl[:, :, 1:3] * SW8_L).astype(FP8)
    return {"wh": wh_scaled, "w8": w8,
            "br": asb(-b_r * SCALE), "bz": asb(-b_z * SCALE),
            "bin": asb(b_in), "bhn": asb(b_hn)}


def prep_x(x_core, KT, BT, scheme=None):
    """x_core: [BL, I, T] fp32 -> per-core input dict."""
    scheme = scheme or SCHEME
    II = KT * P
    if scheme == "f32r":
        # b-major columns (t inner): col = b*4 + t; n-half-major DRAM
        xt = x_core[:, :II, :].transpose(1, 0, 2)      # [I, BL, T]
        xt = xt.reshape(KT, P, 4 * BT).transpose(1, 0, 2)  # [P, KT, TB]
        NT = (4 * BT) // 512
        xt = xt.reshape(P, KT, NT, 512).transpose(2, 0, 1, 3)
        return {"xh": np.ascontiguousarray(xt)}
    xt = x_core[:, :II, :].transpose(1, 2, 0)          # [I, T, BL]
    xt = xt.reshape(KT, P, 4 * BT).transpose(1, 0, 2)  # [P, KT, TB]
    xt = np.ascontiguousarray(xt)
    if scheme == "bf16x3":
        xh = xt.astype(BF16)
        xl = (xt - xh.astype(np.float32)).astype(BF16)
        return {"xh": xh, "xl": xl}
    xh16 = xt.astype(np.float16)
    xl = xt - xh16.astype(np.float32)
    xh_scaled = (xh16.astype(np.float32) * SX_H).astype(np.float16)
    x8 = np.empty((P, KT, 2, 4 * BT), dtype=FP8)
    x8[:, :, 0, :] = (xl * SX8_L).astype(FP8)
    x8[:, :, 1, :] = xh16.astype(np.float32).astype(FP8)
    return {"xh": xh_scaled, "x8": x8}


def unpack_out(out, GJ, BT):
    """out: [GJ, P, 3*BT] fp32 -> spikes [BL, H', 4] with t=0 zeros."""
    HH = GJ * P
    arr = out.reshape(HH, 3, BT)                     # [h, t-1, b]
    res = np.zeros((BT, HH, 4), dtype=np.float32)
    res[:, :, 1:4] = arr.transpose(2, 0, 1)
    return res


def unpack_out_f32r(out, GJ, BT):
    """out: [GJ, NT, P, 3, BN] -> spikes [BL, H', 4] with t=0 zeros."""
    HH = GJ * P
    NT, BN = out.shape[1], out.shape[4]
    arr = out.transpose(1, 4, 0, 2, 3).reshape(NT * BN, HH, 3)
    res = np.zeros((NT * BN, HH, 4), dtype=np.float32)
    res[:, :, 1:4] = arr
    return res


def kernel(inputs, W_ih, b_ih, W_hh, b_hh):
    from concourse.bass_utils import run_bass_kernel_spmd

    # BT = batch rows per timestep per core (= local batch size BL)
    KT, GJ, BT = I // P, H // P, B // NCORES
    key = (KT, GJ, BT, SCHEME)
    if key not in _CACHE:
        _CACHE[key] = build_nc(KT, GJ, BT)
    nc = _CACHE[key]

    wmap = prep_weights(np.asarray(W_ih, dtype=np.float32),
                        np.asarray(b_ih, dtype=np.float32),
                        np.asarray(b_hh, dtype=np.float32), KT, GJ)

    x = np.asarray(inputs, dtype=np.float32)
    in_maps = []
    BL = B // NCORES
    for c in range(NCORES):
        m = dict(wmap)
        m.update(prep_x(x[c * BL:(c + 1) * BL], KT, BT))
        in_maps.append(m)

    res = run_bass_kernel_spmd(nc, in_maps, list(range(NCORES)), trace=TRACE)
    global LAST_EXEC_NS, LAST_RESULTS
    LAST_EXEC_NS = res.exec_time_ns
    LAST_RESULTS = res

    unpack = unpack_out_f32r if SCHEME == "f32r" else unpack_out
    out = np.empty((B, H, T), dtype=np.float32)
    for c in range(NCORES):
        out[c * BL:(c + 1) * BL] = unpack(res.results[c]["out"], GJ, BT)
    return out

